# revision 1
# baseline (speedup 1.0000x reference)
"""CrossAttentionOutLayer Trainium2 kernel.

Math: reference computes, per batch b:
    q = rna @ Wq.T + bq                [n, h*dk]
    k = prot @ Wk.T + bk               [m, h*dk]
    logits[h] = (q_h*scale + rel_h) @ k_h.T
    out = mean_h logits                [n, m]

The head-mean of per-head inner products collapses into one flat inner
product over the h*dk=512 axis:
    out[i,j] = (scale/H * q[i,:] + rel_flat/H) . k[j,:]
so with Wq2 = (scale/H)*Wq, bq2 = (scale/H)*bq + rel_flat/H:
    out = (rna @ Wq2.T + bq2) @ (prot @ Wk.T + bk).T
Three GEMMs per batch. Data-parallel: batch b -> core b (8 cores).

Device schedule (per core):
  - activations shipped feature-major from the host (bf16), weights as
    [in, out] bf16; all GEMMs bf16 with fp32 PSUM accumulation.
  - loop orders chosen so each stationary (lhsT) tile serves 2 matmuls
    (the LDWEIGHTS swap costs ~53ns un-hidden on TRN2; 116 switches vs
    a naive 188).
  - reps>1 (timing mode): software-pipelined ping-pong. The For_i body
    holds TWO logical iterations on alternating SBUF tile sets; each
    half issues the *other* set's DMA loads, so data is always resident
    a half-body (~55us) before its first matmul and the back-edge
    barrier amortizes over 2 reps.
"""

import os

import numpy as np
import ml_dtypes

# A/B experiment knobs (defaults = shipped configuration).
# Reps per loop body (robust estimator): 2 -> 64956 ns, 4 -> 62947,
# 8 -> 61993. Larger bodies amortize the back-edge + boundary cost.
_STAGGER = os.environ.get("KERNEL_STAGGER", "0") == "1"
_BODYREPS = int(os.environ.get("KERNEL_BODYREPS", "8"))
# 1 = emit a standalone ldweights of the NEXT stationary tile between the
# two matmuls of each weight pair (prefetch into the PE weight path)
_XLDW = os.environ.get("KERNEL_XLDW", "0") == "1"
# moving-operand chunk width: 512 or 256 (probe measured N=256 at
# 0.49 ns/col vs 0.54 at N=512 on 8 busy cores)
_NCHUNK = int(os.environ.get("KERNEL_NCHUNK", "512"))
# 1 = one DMA per activation tensor instead of ~1MB chunks (the ping-pong
# lookahead makes chunk-granularity gating unnecessary)
_BIGDMA = os.environ.get("KERNEL_BIGDMA", "0") == "1"
# GEMM3 moving-chunk width (512 or 256). 256 trades 2x matmul count for
# the better per-column pace measured under 8-core load; each chunk gets
# its own (bank-padded) PSUM tile so accumulation groups stay 1:1.
_G3CHUNK = int(os.environ.get("KERNEL_G3CHUNK", "512"))

import concourse.bacc as bacc
import concourse.tile as tile
import concourse.mybir as mybir
from concourse import bass_utils
from concourse.bass import ts

B, N, M = 8, 1024, 1024
DIM2 = 1280            # rna in-features  = 10*128
KIN = 1344             # protein in-features
KINP = 1408            # padded to 11*128
F = 512                # h*dk flat feature dim = 4*128
H, DK = 8, 64
SCALE = DK ** -0.5
NCORES = 8

NQ = DIM2 // 128       # 10 contraction tiles for Q gemm
NK = KINP // 128       # 11 contraction tiles for K gemm
NF = F // 128          # 4 feature tiles
NB = N // 128          # 8 row blocks of output
NMC = M // 512         # 2 column chunks of output

BF16 = mybir.dt.bfloat16
F32 = mybir.dt.float32

_CACHE = {}


def _build_program(reps=1):
    nc = bacc.Bacc(
        "TRN2", target_bir_lowering=False, debug=False, num_devices=NCORES
    )

    rna_d = nc.dram_tensor("rna", [DIM2, N], BF16, kind="ExternalInput").ap()
    prot_d = nc.dram_tensor("prot", [KINP, M], BF16, kind="ExternalInput").ap()
    wq_d = nc.dram_tensor("wqt", [DIM2, F], BF16, kind="ExternalInput").ap()
    wk_d = nc.dram_tensor("wkt", [KINP, F], BF16, kind="ExternalInput").ap()
    b2_d = nc.dram_tensor("b2", [128, 2 * NF], F32, kind="ExternalInput").ap()
    out_d = nc.dram_tensor("out", [N, M], BF16, kind="ExternalOutput").ap()

    nsets = 2 if reps > 1 else 1
    bodyreps = _BODYREPS
    assert reps == 1 or reps % bodyreps == 0, "timing mode needs even reps"

    with tile.TileContext(nc) as tc:
        with (
            tc.tile_pool(name="data", bufs=1) as dpool,
            tc.tile_pool(name="qk", bufs=1) as qkpool,
            tc.tile_pool(name="outs", bufs=4) as opool,
            tc.tile_pool(name="psum", bufs=1, space="PSUM") as pspool,
        ):
            # ---- persistent SBUF tensors (per ping-pong set) ----
            xk = [
                dpool.tile([128, NK, M], BF16, tag=f"xk{s}", name=f"xk{s}")
                for s in range(nsets)
            ]
            xq = [
                dpool.tile([128, NQ, N], BF16, tag=f"xq{s}", name=f"xq{s}")
                for s in range(nsets)
            ]
            wk = [
                dpool.tile([128, NK, F], BF16, tag=f"wk{s}", name=f"wk{s}")
                for s in range(nsets)
            ]
            wq = [
                dpool.tile([128, NQ, F], BF16, tag=f"wq{s}", name=f"wq{s}")
                for s in range(nsets)
            ]
            b2 = [
                dpool.tile([128, 2 * NF], F32, tag=f"b2{s}", name=f"b2{s}")
                for s in range(nsets)
            ]
            # intra-iteration intermediates (single set: on the PE the
            # producer of set s+1 never overtakes the consumer of set s)
            kt = qkpool.tile([128, NF, M], BF16, tag="kt", name="kt")
            q2 = qkpool.tile([128, NF, N], BF16, tag="q2", name="q2")

            def load_set(s):
                # weights on the ACT HWDGE queue; wk first (gates GEMM2)
                nc.scalar.dma_start(
                    wk[s], wk_d.rearrange("(t p) f -> p t f", p=128)
                )
                nc.scalar.dma_start(
                    wq[s], wq_d.rearrange("(t p) f -> p t f", p=128)
                )
                nc.scalar.dma_start(b2[s], b2_d)
                # activations on the SP HWDGE queue, ~1MB chunks so the
                # first GEMM2 matmuls gate on chunk 0 only
                kchunks = ((0, NK),) if _BIGDMA else ((0, 4), (4, 8), (8, NK))
                qchunks = ((0, NQ),) if _BIGDMA else ((0, 4), (4, 8), (8, NQ))
                for lo, hi in kchunks:
                    nc.sync.dma_start(
                        xk[s][:, lo:hi],
                        prot_d[lo * 128 : hi * 128, :].rearrange(
                            "(t p) m -> p t m", p=128
                        ),
                    )
                for lo, hi in qchunks:
                    nc.sync.dma_start(
                        xq[s][:, lo:hi],
                        rna_d[lo * 128 : hi * 128, :].rearrange(
                            "(t p) n -> p t n", p=128
                        ),
                    )

            def compute_set(s, mid_boundary=False):
                # ---- GEMM2: kT[f,m] = sum_i WkT[i,f].T @ protT[i,m] (+bk)
                # contraction-outer: banks interleave, each weight tile
                # (i,f) serves the m-chunks back-to-back. One PSUM tile per
                # (f, chunk) accumulation group (8x512 or 16x256).
                nch = 1024 // _NCHUNK
                nbank = NF * nch
                ps_k = [
                    pspool.tile(
                        [128, _NCHUNK], F32,
                        tag=f"ps{j}", name=f"psk{s}_{j}",
                    )
                    for j in range(nbank)
                ]
                for i in range(NK):
                    for f in range(NF):
                        for c in range(nch):
                            nc.tensor.matmul(
                                ps_k[f * nch + c],
                                wk[s][:, i, ts(f, 128)],
                                xk[s][:, i, c * _NCHUNK : (c + 1) * _NCHUNK],
                                start=(i == 0),
                                stop=(i == NK - 1),
                            )
                for f in range(NF):
                    for c in range(nch):
                        nc.vector.tensor_scalar_add(
                            kt[:, f, c * _NCHUNK : (c + 1) * _NCHUNK],
                            ps_k[f * nch + c],
                            b2[s][:, f : f + 1],
                        )

                if mid_boundary:
                    tc.stage_boundary()

                # ---- GEMM1: q2T[f,n] = sum_i WqT[i,f].T @ rnaT[i,n] (+bq2)
                ps_q = [
                    pspool.tile(
                        [128, _NCHUNK], F32,
                        tag=f"ps{j}", name=f"psq{s}_{j}",
                    )
                    for j in range(nbank)
                ]
                for i in range(NQ):
                    for f in range(NF):
                        for c in range(nch):
                            nc.tensor.matmul(
                                ps_q[f * nch + c],
                                wq[s][:, i, ts(f, 128)],
                                xq[s][:, i, c * _NCHUNK : (c + 1) * _NCHUNK],
                                start=(i == 0),
                                stop=(i == NQ - 1),
                            )
                for f in range(NF):
                    for c in range(nch):
                        nc.vector.tensor_scalar_add(
                            q2[:, f, c * _NCHUNK : (c + 1) * _NCHUNK],
                            ps_q[f * nch + c],
                            b2[s][:, NF + f : NF + f + 1],
                        )

                # ---- GEMM3: out[n,m] = sum_f q2T[f,n].T @ kT[f,m] ----
                # f in the middle: stationary q2 tile (nb,f) serves both
                # m-chunks; two PSUM banks live per nb.
                g3n = _G3CHUNK
                g3c = 1024 // g3n
                for nb in range(NB):
                    ps3 = [
                        pspool.tile(
                            [128, g3n], F32,
                            tag=f"ps{(nb * g3c + c) % nbank}",
                            name=f"ps3_{s}_{nb}_{c}",
                        )
                        for c in range(g3c)
                    ]
                    for f in range(NF):
                        for c in range(g3c):
                            nc.tensor.matmul(
                                ps3[c],
                                q2[:, f, ts(nb, 128)],
                                kt[:, f, c * g3n : (c + 1) * g3n],
                                start=(f == 0),
                                stop=(f == NF - 1),
                            )
                    ot = opool.tile([128, M], BF16, tag="ot", name=f"ot{s}_{nb}")
                    for c in range(g3c):
                        lo = c * g3n
                        if nb % 2 == 0:
                            nc.vector.tensor_copy(ot[:, lo : lo + g3n], ps3[c])
                        else:
                            nc.scalar.activation(
                                ot[:, lo : lo + g3n], ps3[c],
                                mybir.ActivationFunctionType.Copy,
                            )
                    nc.gpsimd.dma_start(out_d[ts(nb, 128), :], ot)

            if reps == 1:
                load_set(0)
                compute_set(0)
            else:
                load_set(0)
                with tc.For_i(
                    0, reps // bodyreps, 1,
                    hint_engines=(mybir.EngineType.PE,),
                    staggered_reset=_STAGGER,
                ):
                    for r in range(bodyreps):
                        load_set((r + 1) % 2)
                        compute_set(r % 2, mid_boundary=_STAGGER)
                        if _STAGGER and r < bodyreps - 1:
                            tc.stage_boundary()

    nc.compile()
    return nc


def _get_program(reps=1):
    key = ("nc", reps)
    if key not in _CACHE:
        _CACHE[key] = _build_program(reps)
    return _CACHE[key]


def _prep_inputs(rna_reps, protein_reps, Wq, bq, Wk, bk, rel_bias):
    bf16 = ml_dtypes.bfloat16
    # fold scale/H into Wq; fold rel_bias head-mean into the q bias
    rel_flat = np.asarray(rel_bias, np.float32).reshape(H * DK)
    wq2t = (np.asarray(Wq, np.float32).T * (SCALE / H)).astype(bf16)  # [DIM2,F]
    bq2 = (SCALE / H) * np.asarray(bq, np.float32) + rel_flat / H
    wkt = np.zeros((KINP, F), dtype=bf16)
    wkt[:KIN] = np.asarray(Wk, np.float32).T.astype(bf16)
    bk2 = np.asarray(bk, np.float32)

    # packed biases: col f -> bk chunk f, col NF+f -> bq chunk f
    b2 = np.empty((128, 2 * NF), np.float32)
    for f in range(NF):
        b2[:, f] = bk2[f * 128 : (f + 1) * 128]
        b2[:, NF + f] = bq2[f * 128 : (f + 1) * 128]

    # feature-major layout: [B, D, tokens]
    rna_bf = (
        np.asarray(rna_reps, np.float32)
        .transpose(0, 2, 1)
        .astype(bf16)
    )
    prot_bf = np.zeros((B, KINP, M), dtype=bf16)
    prot_bf[:, :KIN] = (
        np.asarray(protein_reps, np.float32)
        .transpose(0, 2, 1)
        .astype(bf16)
    )

    in_maps = []
    for b in range(B):
        in_maps.append(
            {
                "rna": np.ascontiguousarray(rna_bf[b]),
                "prot": np.ascontiguousarray(prot_bf[b]),
                "wqt": wq2t,
                "wkt": wkt,
                "b2": b2,
            }
        )
    return in_maps


def kernel(rna_reps, protein_reps, Wq, bq, Wk, bk, rel_bias, **_ignored):
    in_maps = _prep_inputs(rna_reps, protein_reps, Wq, bq, Wk, bk, rel_bias)
    nc = _get_program()
    res = bass_utils.run_bass_kernel_spmd(
        nc, in_maps, core_ids=list(range(NCORES))
    )
    out = np.stack(
        [np.asarray(res.results[b]["out"], np.float32) for b in range(B)], axis=0
    )
    return out



# revision 3
# speedup vs baseline: 1.0152x; 1.0152x over previous
"""CrossAttentionOutLayer Trainium2 kernel.

Math: reference computes, per batch b:
    q = rna @ Wq.T + bq                [n, h*dk]
    k = prot @ Wk.T + bk               [m, h*dk]
    logits[h] = (q_h*scale + rel_h) @ k_h.T
    out = mean_h logits                [n, m]

The head-mean of per-head inner products collapses into one flat inner
product over the h*dk=512 axis:
    out[i,j] = (scale/H * q[i,:] + rel_flat/H) . k[j,:]
so with Wq2 = (scale/H)*Wq, bq2 = (scale/H)*bq + rel_flat/H:
    out = (rna @ Wq2.T + bq2) @ (prot @ Wk.T + bk).T
Three GEMMs per batch. Data-parallel: batch b -> core b (8 cores).

Device schedule (per core):
  - activations shipped feature-major from the host (bf16), weights as
    [in, out] bf16; all GEMMs bf16 with fp32 PSUM accumulation.
  - loop orders chosen so each stationary (lhsT) tile serves 2 matmuls
    (the LDWEIGHTS swap costs ~53ns un-hidden on TRN2; 116 switches vs
    a naive 188).
  - reps>1 (timing mode): software-pipelined ping-pong. The For_i body
    holds TWO logical iterations on alternating SBUF tile sets; each
    half issues the *other* set's DMA loads, so data is always resident
    a half-body (~55us) before its first matmul and the back-edge
    barrier amortizes over 2 reps.
"""

import os

import numpy as np
import ml_dtypes

# A/B experiment knobs (defaults = shipped configuration).
# Reps per loop body (robust estimator): 2 -> 64956 ns, 4 -> 62947,
# 8 -> 61993. Larger bodies amortize the back-edge + boundary cost.
_STAGGER = os.environ.get("KERNEL_STAGGER", "0") == "1"
_BODYREPS = int(os.environ.get("KERNEL_BODYREPS", "8"))
# 1 = emit a standalone ldweights of the NEXT stationary tile between the
# two matmuls of each weight pair (prefetch into the PE weight path)
_XLDW = os.environ.get("KERNEL_XLDW", "0") == "1"
# moving-operand chunk width: 512 or 256 (probe measured N=256 at
# 0.49 ns/col vs 0.54 at N=512 on 8 busy cores)
_NCHUNK = int(os.environ.get("KERNEL_NCHUNK", "512"))
# 1 = one DMA per activation tensor instead of ~1MB chunks (the ping-pong
# lookahead makes chunk-granularity gating unnecessary)
_BIGDMA = os.environ.get("KERNEL_BIGDMA", "0") == "1"
# GEMM3 moving-chunk width (512 or 256). 256 trades 2x matmul count for
# the better per-column pace measured under 8-core load; each chunk gets
# its own (bank-padded) PSUM tile so accumulation groups stay 1:1.
_G3CHUNK = int(os.environ.get("KERNEL_G3CHUNK", "512"))

import concourse.bacc as bacc
import concourse.tile as tile
import concourse.mybir as mybir
from concourse import bass_utils
from concourse.bass import ts

B, N, M = 8, 1024, 1024
DIM2 = 1280            # rna in-features  = 10*128
KIN = 1344             # protein in-features
KINP = 1408            # padded to 11*128
F = 512                # h*dk flat feature dim = 4*128
H, DK = 8, 64
SCALE = DK ** -0.5
NCORES = 8

NQ = DIM2 // 128       # 10 contraction tiles for Q gemm
NK = KINP // 128       # 11 contraction tiles for K gemm
NF = F // 128          # 4 feature tiles
NB = N // 128          # 8 row blocks of output
NMC = M // 512         # 2 column chunks of output

BF16 = mybir.dt.bfloat16
F32 = mybir.dt.float32

_CACHE = {}


def _ldw_sig(inst):
    ap = inst.ins[0]
    mr = getattr(ap, "memref", None)
    return (
        str(getattr(ap, "ap", None)),
        getattr(ap, "offset", None),
        str(getattr(ap, "dtype", None)),
        getattr(mr, "name", None) if mr is not None else repr(ap)[:200],
    )


def _dedupe_ldweights(nc):
    """Drop InstLdweights that reload the exact weights already resident.

    tile_legalize emits one LDWEIGHTS per MATMUL; when one stationary tile
    serves several matmuls back-to-back the repeats are pure PE-serial
    overhead (~53ns each, row-conflict blocks overlap). Safe to remove when
    the repeat carries no semaphore waits/updates and only Matmults ran on
    the PE since the original load. Dangling dep references are remapped to
    the surviving load.
    """
    removed_total = 0
    for fn in nc.m.functions:
        for blk in fn.blocks:
            il = blk.instructions
            keep = []
            remap = {}
            last_sig = None
            last_name = None
            for inst in il:
                if isinstance(inst, mybir.InstLdweights):
                    sig = _ldw_sig(inst)
                    si = inst.sync_info
                    clean = si is None or (not si.on_wait and not si.on_update)
                    if sig == last_sig and clean:
                        remap[inst.name] = last_name
                        removed_total += 1
                        continue
                    last_sig = sig
                    last_name = inst.name
                elif isinstance(inst, mybir.InstMatmult):
                    pass  # weights stay resident
                elif inst.engine == mybir.EngineType.PE:
                    last_sig = None  # other PE inst: be conservative
                keep.append(inst)
            if remap:
                il[:] = keep
                for inst in keep:
                    names = set(inst.sync_dependency_names()) | set(
                        inst.nosync_dependency_names()
                    )
                    if names & set(remap):
                        inst.remap_dependency_names(remap)
    return removed_total


def _build_program(reps=1):
    nc = bacc.Bacc(
        "TRN2", target_bir_lowering=False, debug=False, num_devices=NCORES
    )

    rna_d = nc.dram_tensor("rna", [DIM2, N], BF16, kind="ExternalInput").ap()
    prot_d = nc.dram_tensor("prot", [KINP, M], BF16, kind="ExternalInput").ap()
    wq_d = nc.dram_tensor("wqt", [DIM2, F], BF16, kind="ExternalInput").ap()
    wk_d = nc.dram_tensor("wkt", [KINP, F], BF16, kind="ExternalInput").ap()
    b2_d = nc.dram_tensor("b2", [128, 2 * NF], F32, kind="ExternalInput").ap()
    out_d = nc.dram_tensor("out", [N, M], BF16, kind="ExternalOutput").ap()

    nsets = 2 if reps > 1 else 1
    bodyreps = _BODYREPS
    assert reps == 1 or reps % bodyreps == 0, "timing mode needs even reps"

    with tile.TileContext(nc) as tc:
        with (
            tc.tile_pool(name="data", bufs=1) as dpool,
            tc.tile_pool(name="qk", bufs=1) as qkpool,
            tc.tile_pool(name="outs", bufs=4) as opool,
            tc.tile_pool(name="psum", bufs=1, space="PSUM") as pspool,
        ):
            # ---- persistent SBUF tensors (per ping-pong set) ----
            xk = [
                dpool.tile([128, NK, M], BF16, tag=f"xk{s}", name=f"xk{s}")
                for s in range(nsets)
            ]
            xq = [
                dpool.tile([128, NQ, N], BF16, tag=f"xq{s}", name=f"xq{s}")
                for s in range(nsets)
            ]
            wk = [
                dpool.tile([128, NK, F], BF16, tag=f"wk{s}", name=f"wk{s}")
                for s in range(nsets)
            ]
            wq = [
                dpool.tile([128, NQ, F], BF16, tag=f"wq{s}", name=f"wq{s}")
                for s in range(nsets)
            ]
            b2 = [
                dpool.tile([128, 2 * NF], F32, tag=f"b2{s}", name=f"b2{s}")
                for s in range(nsets)
            ]
            # intra-iteration intermediates (single set: on the PE the
            # producer of set s+1 never overtakes the consumer of set s)
            kt = qkpool.tile([128, NF, M], BF16, tag="kt", name="kt")
            q2 = qkpool.tile([128, NF, N], BF16, tag="q2", name="q2")

            def load_set(s):
                # weights on the ACT HWDGE queue; wk first (gates GEMM2)
                nc.scalar.dma_start(
                    wk[s], wk_d.rearrange("(t p) f -> p t f", p=128)
                )
                nc.scalar.dma_start(
                    wq[s], wq_d.rearrange("(t p) f -> p t f", p=128)
                )
                nc.scalar.dma_start(b2[s], b2_d)
                # activations on the SP HWDGE queue, ~1MB chunks so the
                # first GEMM2 matmuls gate on chunk 0 only
                kchunks = ((0, NK),) if _BIGDMA else ((0, 4), (4, 8), (8, NK))
                qchunks = ((0, NQ),) if _BIGDMA else ((0, 4), (4, 8), (8, NQ))
                for lo, hi in kchunks:
                    nc.sync.dma_start(
                        xk[s][:, lo:hi],
                        prot_d[lo * 128 : hi * 128, :].rearrange(
                            "(t p) m -> p t m", p=128
                        ),
                    )
                for lo, hi in qchunks:
                    nc.sync.dma_start(
                        xq[s][:, lo:hi],
                        rna_d[lo * 128 : hi * 128, :].rearrange(
                            "(t p) n -> p t n", p=128
                        ),
                    )

            def compute_set(s, mid_boundary=False):
                # ---- GEMM2: kT[f,m] = sum_i WkT[i,f].T @ protT[i,m] (+bk)
                # contraction-outer: banks interleave, each weight tile
                # (i,f) serves the m-chunks back-to-back. One PSUM tile per
                # (f, chunk) accumulation group (8x512 or 16x256).
                nch = 1024 // _NCHUNK
                nbank = NF * nch
                ps_k = [
                    pspool.tile(
                        [128, _NCHUNK], F32,
                        tag=f"ps{j}", name=f"psk{s}_{j}",
                    )
                    for j in range(nbank)
                ]
                for i in range(NK):
                    for f in range(NF):
                        for c in range(nch):
                            nc.tensor.matmul(
                                ps_k[f * nch + c],
                                wk[s][:, i, ts(f, 128)],
                                xk[s][:, i, c * _NCHUNK : (c + 1) * _NCHUNK],
                                start=(i == 0),
                                stop=(i == NK - 1),
                            )
                for f in range(NF):
                    for c in range(nch):
                        nc.vector.tensor_scalar_add(
                            kt[:, f, c * _NCHUNK : (c + 1) * _NCHUNK],
                            ps_k[f * nch + c],
                            b2[s][:, f : f + 1],
                        )

                if mid_boundary:
                    tc.stage_boundary()

                # ---- GEMM1: q2T[f,n] = sum_i WqT[i,f].T @ rnaT[i,n] (+bq2)
                ps_q = [
                    pspool.tile(
                        [128, _NCHUNK], F32,
                        tag=f"ps{j}", name=f"psq{s}_{j}",
                    )
                    for j in range(nbank)
                ]
                for i in range(NQ):
                    for f in range(NF):
                        for c in range(nch):
                            nc.tensor.matmul(
                                ps_q[f * nch + c],
                                wq[s][:, i, ts(f, 128)],
                                xq[s][:, i, c * _NCHUNK : (c + 1) * _NCHUNK],
                                start=(i == 0),
                                stop=(i == NQ - 1),
                            )
                for f in range(NF):
                    for c in range(nch):
                        nc.vector.tensor_scalar_add(
                            q2[:, f, c * _NCHUNK : (c + 1) * _NCHUNK],
                            ps_q[f * nch + c],
                            b2[s][:, NF + f : NF + f + 1],
                        )

                # ---- GEMM3: out[n,m] = sum_f q2T[f,n].T @ kT[f,m] ----
                # f in the middle: stationary q2 tile (nb,f) serves both
                # m-chunks; two PSUM banks live per nb.
                g3n = _G3CHUNK
                g3c = 1024 // g3n
                for nb in range(NB):
                    ps3 = [
                        pspool.tile(
                            [128, g3n], F32,
                            tag=f"ps{(nb * g3c + c) % nbank}",
                            name=f"ps3_{s}_{nb}_{c}",
                        )
                        for c in range(g3c)
                    ]
                    for f in range(NF):
                        for c in range(g3c):
                            nc.tensor.matmul(
                                ps3[c],
                                q2[:, f, ts(nb, 128)],
                                kt[:, f, c * g3n : (c + 1) * g3n],
                                start=(f == 0),
                                stop=(f == NF - 1),
                            )
                    ot = opool.tile([128, M], BF16, tag="ot", name=f"ot{s}_{nb}")
                    for c in range(g3c):
                        lo = c * g3n
                        if nb % 2 == 0:
                            nc.vector.tensor_copy(ot[:, lo : lo + g3n], ps3[c])
                        else:
                            nc.scalar.activation(
                                ot[:, lo : lo + g3n], ps3[c],
                                mybir.ActivationFunctionType.Copy,
                            )
                    nc.gpsimd.dma_start(out_d[ts(nb, 128), :], ot)

            if reps == 1:
                load_set(0)
                compute_set(0)
            else:
                load_set(0)
                with tc.For_i(
                    0, reps // bodyreps, 1,
                    hint_engines=(mybir.EngineType.PE,),
                    staggered_reset=_STAGGER,
                ):
                    for r in range(bodyreps):
                        load_set((r + 1) % 2)
                        compute_set(r % 2, mid_boundary=_STAGGER)
                        if _STAGGER and r < bodyreps - 1:
                            tc.stage_boundary()

    _dedupe_ldweights(nc)
    nc.compile()
    return nc


def _get_program(reps=1):
    key = ("nc", reps)
    if key not in _CACHE:
        _CACHE[key] = _build_program(reps)
    return _CACHE[key]


def _prep_inputs(rna_reps, protein_reps, Wq, bq, Wk, bk, rel_bias):
    bf16 = ml_dtypes.bfloat16
    # fold scale/H into Wq; fold rel_bias head-mean into the q bias
    rel_flat = np.asarray(rel_bias, np.float32).reshape(H * DK)
    wq2t = (np.asarray(Wq, np.float32).T * (SCALE / H)).astype(bf16)  # [DIM2,F]
    bq2 = (SCALE / H) * np.asarray(bq, np.float32) + rel_flat / H
    wkt = np.zeros((KINP, F), dtype=bf16)
    wkt[:KIN] = np.asarray(Wk, np.float32).T.astype(bf16)
    bk2 = np.asarray(bk, np.float32)

    # packed biases: col f -> bk chunk f, col NF+f -> bq chunk f
    b2 = np.empty((128, 2 * NF), np.float32)
    for f in range(NF):
        b2[:, f] = bk2[f * 128 : (f + 1) * 128]
        b2[:, NF + f] = bq2[f * 128 : (f + 1) * 128]

    # feature-major layout: [B, D, tokens]
    rna_bf = (
        np.asarray(rna_reps, np.float32)
        .transpose(0, 2, 1)
        .astype(bf16)
    )
    prot_bf = np.zeros((B, KINP, M), dtype=bf16)
    prot_bf[:, :KIN] = (
        np.asarray(protein_reps, np.float32)
        .transpose(0, 2, 1)
        .astype(bf16)
    )

    in_maps = []
    for b in range(B):
        in_maps.append(
            {
                "rna": np.ascontiguousarray(rna_bf[b]),
                "prot": np.ascontiguousarray(prot_bf[b]),
                "wqt": wq2t,
                "wkt": wkt,
                "b2": b2,
            }
        )
    return in_maps


def kernel(rna_reps, protein_reps, Wq, bq, Wk, bk, rel_bias, **_ignored):
    in_maps = _prep_inputs(rna_reps, protein_reps, Wq, bq, Wk, bk, rel_bias)
    nc = _get_program()
    res = bass_utils.run_bass_kernel_spmd(
        nc, in_maps, core_ids=list(range(NCORES))
    )
    out = np.stack(
        [np.asarray(res.results[b]["out"], np.float32) for b in range(B)], axis=0
    )
    return out

